# revision 12
# baseline (speedup 1.0000x reference)
"""Trainium2 Bass kernel for dense multi-head attention (b=2, n=2048, dim=1024, h=16, dh=64).

Sharding: tensor-parallel over heads -- 2 heads per NeuronCore x 8 cores.
Each core computes QKV projection for its heads, attention, and a partial
output projection (w_out input-dim slice); partials are summed on host.

Device-side layout choices:
  - All TensorE matmuls in bf16 (1 cycle/row at any N).
  - qT/kT kept head-transposed [dh, tokens]; V in token-major [tokens, dh]
    with a ones column appended so P@V also yields the softmax row sums.
  - softmax uses p = exp(S) * exp(bias); exp(bias) precomputed on host
    (mask folded in multiplicatively: masked -> 0).
  - p transposed 128x128 via TensorE for the P@V contraction.
  - 1/s computed as exp(-log(s)) on ScalarE (keeps a single ACT table set).
"""

import numpy as np
import ml_dtypes

import concourse.bass as bass
import concourse.tile as tile
from concourse import bacc
from concourse import mybir
from concourse.bass_utils import run_bass_kernel_spmd
from concourse.masks import make_identity

BF16 = mybir.dt.bfloat16
F32 = mybir.dt.float32
NPBF16 = ml_dtypes.bfloat16

B, N, DIM, HEADS, DH = 2, 2048, 1024, 16, 64
T = B * N  # 4096 tokens total
HPC = 2    # heads per core
NCORES = 8
SCALE = DH ** -0.5
EXP = mybir.ActivationFunctionType.Exp
LOG = mybir.ActivationFunctionType.Ln
MULT = mybir.AluOpType.mult


def _install_trace_hook():
    """Shim antenv.axon_hooks so run_bass_kernel_spmd(trace=True) can capture
    NTFF profiles via the axon .so (the agent image's antenv lacks the module)."""
    import sys
    import types

    try:
        import antenv

        if "antenv.axon_hooks" in sys.modules:
            return
        mod = types.ModuleType("antenv.axon_hooks")
        mod._HOOK = None
        mod.set_axon_ntff_profile_hook = lambda h: setattr(mod, "_HOOK", h)
        mod.get_axon_ntff_profile_hook = lambda: mod._HOOK
        sys.modules["antenv.axon_hooks"] = mod
        antenv.axon_hooks = mod
        from trn_agent_boot.trn_boot import _ntff_profile_via_ctypes

        hook = _ntff_profile_via_ctypes("/opt/axon/libaxon_pjrt.so")
        if hook is not None:
            mod._HOOK = hook
    except Exception:
        pass


_install_trace_hook()


def build_nc():
    nc = bacc.Bacc()
    xT = nc.declare_dram_parameter("xT", [DIM, T], BF16, isOutput=False)
    wT = nc.declare_dram_parameter("wT", [DIM, 3 * HPC * DH], BF16, isOutput=False)
    wo0 = nc.declare_dram_parameter("wo0", [DH, DIM], BF16, isOutput=False)
    wo1 = nc.declare_dram_parameter("wo1", [DH, DIM], BF16, isOutput=False)
    EBT = nc.declare_dram_parameter("EBT", [B * HPC, N, N], BF16, isOutput=False)
    out = nc.declare_dram_parameter("out", [T, DIM], BF16, isOutput=True)

    with tile.TileContext(nc) as tc:
        with (
            tc.tile_pool(name="singles", bufs=1) as singles,
            tc.tile_pool(name="ebt", bufs=2) as ebtpool,
            tc.tile_pool(name="p0", bufs=3) as p0pool,
            tc.tile_pool(name="pt", bufs=2) as ptpool,
            tc.tile_pool(name="s64", bufs=2) as s64pool,
            tc.tile_pool(name="ysb", bufs=2) as ysbpool,
            tc.tile_pool(name="ps", bufs=2, space="PSUM") as pspool,
            tc.tile_pool(name="tp", bufs=2, space="PSUM") as tppool,
            tc.tile_pool(name="av", bufs=2, space="PSUM") as avpool,
        ):
            # ---- persistent SBUF ----
            xT_sb = singles.tile([128, 8, T], BF16)        # [d%128, d//128, t]
            wT_sb = singles.tile([128, 8, 3 * HPC * DH], BF16)
            wo_sb0 = singles.tile([DH, DIM], BF16)
            wo_sb1 = singles.tile([DH, DIM], BF16)
            ident = singles.tile([128, 128], BF16)
            qT_sb = singles.tile([128, T], BF16)           # rows: h0 q (64) | h1 q (64)
            kT_sb = singles.tile([128, T], BF16)
            V_sb = singles.tile([128, B * HPC, 16, DH + 1], BF16)  # [j%128, bh, j//128, d|ones]
            OTr = [singles.tile([DH, T], BF16, tag=f"otr{h}", name=f"otr{h}") for h in range(HPC)]
            RBC = [singles.tile([DH, T], BF16, tag=f"rbc{h}", name=f"rbc{h}") for h in range(HPC)]

            ones64 = singles.tile([DH + 1, DH], BF16)

            make_identity(nc, ident)
            nc.vector.memset(V_sb[:, :, :, DH : DH + 1], 1.0)
            nc.vector.memset(ones64, 1.0)

            nc.sync.dma_start(out=xT_sb, in_=xT.rearrange("(dc p) t -> p dc t", p=128))
            nc.sync.dma_start(out=wT_sb, in_=wT.rearrange("(dc p) e -> p dc e", p=128))
            nc.sync.dma_start(out=wo_sb0, in_=wo0[:, :])
            nc.sync.dma_start(out=wo_sb1, in_=wo1[:, :])

            # ---- Q/K projection: qkvT[e, t] = sum_d w[e, d] * x[t, d] ----
            for eg in range(2):  # 0=q, 1=k
                dst = qT_sb if eg == 0 else kT_sb
                for tc8 in range(8):
                    ps = pspool.tile([128, 1024], F32, tag="spsum")
                    for dc in range(8):
                        nc.tensor.matmul(
                            ps[:, :512],
                            lhsT=wT_sb[:, dc, eg * 128 : (eg + 1) * 128],
                            rhs=xT_sb[:, dc, tc8 * 512 : (tc8 + 1) * 512],
                            start=(dc == 0),
                            stop=(dc == 7),
                        )
                    nc.vector.tensor_copy(
                        out=dst[:, tc8 * 512 : (tc8 + 1) * 512], in_=ps[:, :512]
                    )

            # ---- V projection in token-major: V[t, e] ----
            for tt in range(32):
                ps = pspool.tile([128, 1024], F32, tag="spsum")
                for dc in range(8):
                    nc.tensor.matmul(
                        ps[:, :128],
                        lhsT=xT_sb[:, dc, tt * 128 : (tt + 1) * 128],
                        rhs=wT_sb[:, dc, 256:384],
                        start=(dc == 0),
                        stop=(dc == 7),
                    )
                b, jc = tt // 16, tt % 16
                nc.vector.tensor_copy(
                    out=V_sb[:, b * HPC : (b + 1) * HPC, jc, 0:DH],
                    in_=ps[:, :128].rearrange("p (h d) -> p h d", d=DH),
                )

            # ---- attention per (batch, local head) ----
            for bh in range(B * HPC):
                b, hl = bh // HPC, bh % HPC
                e0 = hl * DH  # partition base of this head's qT/kT rows
                for ic in range(4):  # i-chunks of 512
                    pT = ptpool.tile([128, 16, 512], BF16, tag="pT")
                    ebts = []
                    for jh in range(2):
                        ebt = ebtpool.tile([128, 8, 512], BF16, tag="ebt", name=f"ebt{jh}")
                        nc.sync.dma_start(
                            out=ebt,
                            in_=EBT[
                                bh, jh * 1024 : (jh + 1) * 1024, ic * 512 : (ic + 1) * 512
                            ].rearrange("(jb p) i -> p jb i", p=128),
                        )
                        ebts.append(ebt)
                    for isub in range(4):
                        i0 = ic * 512 + isub * 128
                        for jh in range(2):  # j halves of 1024
                            s_ps = pspool.tile([128, 1024], F32, tag="spsum")
                            for nn in range(2):
                                j0 = jh * 1024 + nn * 512
                                nc.tensor.matmul(
                                    s_ps[:, nn * 512 : (nn + 1) * 512],
                                    lhsT=qT_sb[e0 : e0 + DH, b * N + i0 : b * N + i0 + 128],
                                    rhs=kT_sb[e0 : e0 + DH, b * N + j0 : b * N + j0 + 512],
                                    start=True,
                                    stop=True,
                                )
                            p0 = p0pool.tile([128, 1024], BF16, tag="p0")
                            nc.scalar.activation(p0, s_ps, EXP)
                            tp = tppool.tile([128, 1024], BF16, tag="tpsum")
                            for jb in range(8):
                                nc.tensor.transpose(
                                    tp[:, jb * 128 : (jb + 1) * 128],
                                    p0[:, jb * 128 : (jb + 1) * 128],
                                    ident,
                                )
                            # fused: pT = (exp(S))^T * exp(bias)^T  (PSUM read + EB mult + SBUF write)
                            nc.vector.tensor_tensor(
                                pT[:, jh * 8 : (jh + 1) * 8, isub * 128 : (isub + 1) * 128],
                                tp.rearrange("p (jb i) -> p jb i", i=128),
                                ebts[jh][:, :, isub * 128 : (isub + 1) * 128],
                                MULT,
                            )
                    # O^T[d, i] (+ row DH = softmax sums) = sum_j [V|1][j, d] p[i, j]
                    av = avpool.tile([DH + 1, 512], F32, tag="av")
                    for jc in range(16):
                        nc.tensor.matmul(
                            av,
                            lhsT=V_sb[:, bh, jc, :],
                            rhs=pT[:, jc, :],
                            start=(jc == 0),
                            stop=(jc == 15),
                        )
                    tslice = slice(b * N + ic * 512, b * N + (ic + 1) * 512)
                    nc.vector.tensor_copy(out=OTr[hl][:, tslice], in_=av[0:DH])
                    # 1/s = exp(-log(s)); s lives on partition row DH(=64)
                    s64 = s64pool.tile([DH + 1, 512], F32, tag="s64")
                    nc.vector.tensor_copy(out=s64[DH : DH + 1], in_=av[DH : DH + 1])
                    nc.scalar.activation(s64[DH : DH + 1], s64[DH : DH + 1], LOG)
                    r64 = s64pool.tile([DH + 1, 512], BF16, tag="r64")
                    nc.scalar.activation(r64[DH : DH + 1], s64[DH : DH + 1], EXP, scale=-1.0)
                    # broadcast 1/s across the 64 e-partitions via K=1 matmul
                    bc = avpool.tile([DH + 1, 512], F32, tag="av", name="bc")
                    nc.tensor.matmul(
                        bc[0:DH, :],
                        lhsT=ones64[DH : DH + 1, :],
                        rhs=r64[DH : DH + 1, :],
                        start=True,
                        stop=True,
                    )
                    nc.vector.tensor_copy(out=RBC[hl][:, tslice], in_=bc[0:DH, :])

            # ---- normalize in place: OTr *= (1/s) ----
            for hl in range(HPC):
                nc.vector.tensor_tensor(OTr[hl], OTr[hl], RBC[hl], MULT)

            # ---- output projection partial: y[t, dout] = sum_e OTn[e, t] wo[e, dout] ----
            for tt in range(32):
                yt = ysbpool.tile([128, 1024], BF16, tag="ysb")
                for dc2 in range(2):
                    ps = pspool.tile([128, 1024], F32, tag="spsum")
                    nc.tensor.matmul(
                        ps[:, :512],
                        lhsT=OTr[0][:, tt * 128 : (tt + 1) * 128],
                        rhs=wo_sb0[:, dc2 * 512 : (dc2 + 1) * 512],
                        start=True,
                        stop=False,
                    )
                    nc.tensor.matmul(
                        ps[:, :512],
                        lhsT=OTr[1][:, tt * 128 : (tt + 1) * 128],
                        rhs=wo_sb1[:, dc2 * 512 : (dc2 + 1) * 512],
                        start=False,
                        stop=True,
                    )
                    nc.vector.tensor_copy(
                        out=yt[:, dc2 * 512 : (dc2 + 1) * 512], in_=ps[:, :512]
                    )
                nc.sync.dma_start(out=out[tt * 128 : (tt + 1) * 128, :], in_=yt)

    return nc


_NC = None


def _get_nc():
    global _NC
    if _NC is None:
        _NC = build_nc()
        _NC.finalize()
    return _NC


def prepare_in_maps(x, mask, attn_bias, w_qkv, w_out, b_out):
    x = np.asarray(x, np.float32)
    mask = np.asarray(mask)
    attn_bias = np.asarray(attn_bias, np.float32)
    w_qkv = np.asarray(w_qkv, np.float32)
    w_out = np.asarray(w_out, np.float32)
    if not mask.all():
        attn_bias = np.where(mask[:, None, None, :], attn_bias, -np.inf)
    # exp(bias), transposed to [b, h, j, i]; masked -> 0 (multiplicative mask)
    EBT_full = np.exp(attn_bias).transpose(0, 1, 3, 2).astype(NPBF16)
    xT = np.ascontiguousarray(x.reshape(T, DIM).T).astype(NPBF16)
    inner = HEADS * DH
    wq, wk, wv = w_qkv[:inner], w_qkv[inner : 2 * inner], w_qkv[2 * inner :]
    in_maps = []
    for c in range(NCORES):
        sl = slice(HPC * c * DH, HPC * (c + 1) * DH)
        wstack = np.concatenate([wq[sl] * SCALE, wk[sl], wv[sl]], axis=0)  # [384, 1024]
        wT_c = np.ascontiguousarray(wstack.T).astype(NPBF16)
        wo_c = w_out[:, sl]  # [1024, 128]
        wo0 = np.ascontiguousarray(wo_c[:, :DH].T).astype(NPBF16)
        wo1 = np.ascontiguousarray(wo_c[:, DH:].T).astype(NPBF16)
        ebc = EBT_full[:, HPC * c : HPC * (c + 1)].reshape(B * HPC, N, N)
        in_maps.append({"xT": xT, "wT": wT_c, "wo0": wo0, "wo1": wo1, "EBT": ebc})
    return in_maps


def run_device(in_maps, **kwargs):
    return run_bass_kernel_spmd(_get_nc(), in_maps, core_ids=list(range(NCORES)), **kwargs)


def finish(results, b_out):
    y = np.zeros((T, DIM), np.float32)
    for r in results:
        y += np.asarray(r["out"], np.float32)
    y += np.asarray(b_out, np.float32)[None, :]
    return y.reshape(B, N, DIM).astype(np.float32)


def kernel(x, mask, attn_bias, w_qkv, w_out, b_out):
    in_maps = prepare_in_maps(x, mask, attn_bias, w_qkv, w_out, b_out)
    res = run_device(in_maps)
    return finish(res.results, b_out)
